# revision 1
# baseline (speedup 1.0000x reference)
"""Trainium2 Bass kernel for BatchedStarNetAttentionBlock.

Strategy: data-parallel over ordering segments (attention is block-diagonal,
never crosses segment boundaries). Each of the 8 cores gets a subset of
segments, padded to a shared static structure so one SPMD program serves all
cores. No collectives.

On-device layout: activations transposed, xT[d, n], feature dim on partitions
(2 tiles of 128). Heads live at partition offsets 32*hh inside each 128-half,
so score matmuls read qT/kT slices directly with tile_position row groups (no
SBUF->SBUF shifts). LayerNorm uses the centered-weight trick: LN outputs are
exactly zero-mean (g=1,b=0), so the residual stream is always column-centered
and the output projection uses host-centered WoC; the residual is added into
the same PSUM accumulation via an identity matmul, leaving z = y - mean(y) in
PSUM with no separate stats pass. Warm-up matmuls run during the input DMA to
hold the PE clock at full rate.
"""

import sys

for _p in ("/opt/trn_rl_repo",):
    if _p not in sys.path:
        sys.path.insert(0, _p)

import numpy as np
import ml_dtypes

import bass_rust as _bass_rust

import concourse.bass as bass
import concourse.tile as tile
from concourse import bacc
from concourse import mybir
from concourse.bass_utils import run_bass_kernel_spmd
from concourse.hw_specs import get_activation_tables


class _Bacc(bacc.Bacc):
    """Bacc whose activation-table planner gives Tanh its own set (position 0)
    and funnels exp/ln/square/identity/copy into natural_log_exp_and_others,
    so the whole kernel pays exactly one mid-kernel table switch (after the
    block-0 tanh) instead of ping-ponging."""

    def insert_act_table_loads(self):
        has_activation = any(
            isinstance(i, mybir.InstActivation)
            for b in self.main_func.blocks
            for i in b.instructions
        )
        if not has_activation:
            return
        tables = list(get_activation_tables(self.m.arch).items())
        pref = "natural_log_exp_and_others"
        TANH = mybir.ActivationFunctionType.Tanh
        doctored = []
        seen_pref = False
        for name, fns in tables:
            if name == pref:
                seen_pref = True
                doctored.append((name, fns))
            elif not seen_pref:
                doctored.append((name, {TANH} & fns))
            else:
                doctored.append((name, fns))
        _bass_rust.insert_act_table_loads(self, doctored)


P = 128
D = 256
H = 8
DH = 32
SCALE = 1.0 / float(np.sqrt(DH))
N_CORES = 8
NEG = -1e9

F32 = mybir.dt.float32
BF16 = mybir.dt.bfloat16

DT_ACT_NAME = "bf16"

# bisect switches
USE_WARMUP = True       # PE warm-up matmuls during input DMA
USE_GPSIMD_SQ = False    # z^2 on the Pool engine instead of DVE
SCORES_MODE = "maskedk"


def _np_act():
    return ml_dtypes.bfloat16 if DT_ACT_NAME == "bf16" else np.float32


# ---------------------------------------------------------------------------
# weight packing layout (shared between host packer and device program)
# ---------------------------------------------------------------------------
# W_all [128, n_wcols] bf16: matmul weight chunks, 128 cols each.
#   chunk_col(base, k, j) = base + k*(2*128) + j*128   (k-outer, j-inner)
#   lin_W at 0 (4 chunks); id128 at 512; blocks at 640 + i*2048
#   bo rows (bias, [1,128] per (i,j)) at 4736
LIN_BASE = 0
ID_BASE = 512
BLK_BASE = 640
BO_BASE = 4736
N_WCOLS = 4736 + 512


def w_base(i, which):
    return BLK_BASE + i * 2048 + {"q": 0, "k": 512, "v": 1024, "o": 1536}[which]


# C_all [128, n_ccols] f32: per-feature columns (partition = feature within
# d-tile j).
#   0,1   lin_b (j=0,1)
#   2,3   lin_g
#   4,5   lin_beta
#   6+i*12 + [0,1]=bq, [2,3]=bk, [4,5]=bv, [6,7]=bo, [8,9]=ln_g, [10,11]=ln_b
#   30..30+T  maskbias columns (per key-tile)
def c_lin(which, j):
    return {"b": 0, "g": 2, "beta": 4}[which] + j


def c_blk(i, which, j):
    return 6 + i * 12 + {"q": 0, "k": 2, "v": 4, "o": 6, "g": 8, "beta": 10}[which] + j


C_HMASK = 30
C_FIXED = 34


# ---------------------------------------------------------------------------
# device program
# ---------------------------------------------------------------------------
def build_program(slot_ts, trivial_ln, trivial_b):
    T = int(sum(slot_ts))
    NC = T * P          # padded node count per core
    CHW = 256           # chunk width for projections / elementwise stages
    NCH = [(c0, min(CHW, NC - c0)) for c0 in range(0, NC, CHW)]
    GRW = 512           # group width for LN row stages (psum-bank bound)
    NGR = [(g0, min(GRW, NC - g0)) for g0 in range(0, NC, GRW)]

    nc = _Bacc()
    featT = nc.declare_dram_parameter("featT", [P, 2, NC], BF16, isOutput=False)
    wall = nc.declare_dram_parameter("wall", [P, N_WCOLS], BF16, isOutput=False)
    cons = nc.declare_dram_parameter("cons", [P, C_FIXED + T], F32, isOutput=False)
    outT = nc.declare_dram_parameter("outT", [P, 2, NC], F32, isOutput=True)

    # global q-tile index -> (slot, local tile, col offset); slot key tiles
    qtiles = []
    t_off = 0
    for s, ts_s in enumerate(slot_ts):
        for lt in range(ts_s):
            qtiles.append((s, t_off, ts_s))
        t_off += ts_s
    n_qt = len(qtiles)

    with tile.TileContext(nc) as tc:
        with (
            tc.tile_pool(name="wp", bufs=1) as wp,
            tc.tile_pool(name="xp", bufs=1) as xp,
            tc.tile_pool(name="pp", bufs=max(4, 2 * max(slot_ts) + 2)) as pp,
            tc.tile_pool(name="rows", bufs=4) as rows,
            tc.tile_pool(name="ps1", bufs=4, space="PSUM") as ps1,
            tc.tile_pool(name="ps2", bufs=2, space="PSUM") as ps2,
        ):
            # ---- constants (no DMA deps) ----
            ones32 = wp.tile([P, 32], BF16, tag="ones32")
            nc.vector.memset(ones32, 1.0)
            ones_row = wp.tile([1, P], BF16, tag="ones_row")
            nc.vector.memset(ones_row, 1.0)
            neg_row = wp.tile([1, P], BF16, tag="neg_row")
            nc.vector.memset(neg_row, -1.0)
            c256 = wp.tile([P, 1], BF16, tag="c256")
            nc.vector.memset(c256, 1.0 / 256.0)
            eps_row = wp.tile([1, 1], F32, tag="eps_row")
            nc.vector.memset(eps_row, 1e-5)

            # ---- input DMAs, parallel queues; critical loads first ----
            x0 = xp.tile([P, 2, NC], BF16, tag="x0", name="x0")
            nc.sync.dma_start(x0[:], featT[:])
            w_lin = wp.tile([P, 640], BF16, tag="w_lin")  # lin chunks + id128
            nc.scalar.dma_start(w_lin[:], wall[:, 0:640])
            w_blk = [wp.tile([P, 2048], BF16, tag=f"w_blk{i}", name=f"w_blk{i}")
                     for i in range(2)]
            nc.gpsimd.dma_start(w_blk[0][:], wall[:, BLK_BASE:BLK_BASE + 2048])
            c_sb = wp.tile([P, C_FIXED + T], F32, tag="c")
            nc.sync.dma_start(c_sb[:], cons[:])
            nc.gpsimd.dma_start(w_blk[1][:],
                                wall[:, BLK_BASE + 2048:BLK_BASE + 4096])
            bo_rows = wp.tile([1, 512], BF16, tag="bo_rows")
            if not trivial_b:
                nc.gpsimd.dma_start(bo_rows[:], wall[0:1, BO_BASE:BO_BASE + 512])

            def wcol(base, k, j, width=P):
                if base < 640:
                    wt, rel = w_lin, base
                else:
                    i = (base - BLK_BASE) // 2048
                    wt, rel = w_blk[i], (base - BLK_BASE) % 2048
                c0 = rel + k * 256 + j * 128
                return wt[:, c0:c0 + width]

            def id128():
                return w_lin[:, ID_BASE:ID_BASE + 128]

            def ccol(idx):
                return c_sb[:, idx:idx + 1]

            def repN(ap, n):
                # insert a stride-0 dim of size n after the partition dim
                return bass.AP(
                    tensor=ap.tensor, offset=ap.offset,
                    ap=[list(ap.ap[0]), [0, n]] + [list(a) for a in ap.ap[1:]],
                )

            def ones_wide(cw):
                # [1, cw] all-ones view (stride-0 over one element)
                return bass.AP(tensor=ones_row.tensor, offset=ones_row.offset,
                               ap=[list(ones_row.ap[0]), [0, cw]])

            # ---- PE warm-up: keep HAM busy while inputs stream in ----
            warm_rhs_big = repN(ones32[:], 16)   # [P, 16, 32] -> N=512

            def keepalive(n):
                """dummy matmuls in the PE stream to hold HAM at K=8/8
                across a known PE-wait; uses the stp ring (idle there).
                Measured: holds the clock warm (throttle 52us -> 29us) but
                the critical path is ACT/DVE, so wall time got worse; off."""
                return
                if n <= 0:
                    return
                ka = ps2.tile([32, 512], F32, tag="stp", name="ka")
                for w in range(n):
                    nc.tensor.matmul(ka[:, :], ones32[:], warm_rhs_big,
                                     start=(w == 0), stop=(w == n - 1))

            if USE_WARMUP:
                warm_ps = ps1.tile([32, 512], F32, tag="p1", name="warm")
                warm_rhs_sm = repN(ones32[:], 4)     # N=128
                NW_BIG, NW_SM = 8, 8
                for w in range(NW_BIG):
                    nc.tensor.matmul(warm_ps[:, :], ones32[:], warm_rhs_big,
                                     start=(w == 0), stop=False)
                for w in range(NW_SM):
                    nc.tensor.matmul(warm_ps[:, 0:128], ones32[:], warm_rhs_sm,
                                     start=False, stop=(w == NW_SM - 1))

            # ---------------------------------------------------------------
            # shared helpers
            # ---------------------------------------------------------------
            def proj_chunks(src, base, bias_idx, out_tag, copy_eng="dve"):
                """pair projection -> SBUF bf16 [P, 2, NC]; chunked."""
                out = xp.tile([P, 2, NC], BF16, tag=out_tag, name=out_tag)
                for c0, cw in NCH:
                    ps = ps1.tile([P, 2, cw], F32, tag="p1", name=f"pj_{out_tag}")
                    for j in range(2):
                        for k in range(2):
                            nc.tensor.matmul(
                                ps[:, j, :],
                                wcol(base, k, j),
                                src[:, k, c0:c0 + cw],
                                start=(k == 0), stop=(k == 1),
                            )
                    dst = out[:, :, c0:c0 + cw]
                    if trivial_b:
                        if copy_eng == "act":
                            nc.scalar.activation(
                                dst, ps, mybir.ActivationFunctionType.Copy)
                        else:
                            nc.vector.tensor_copy(dst, ps)
                    else:
                        for j in range(2):
                            nc.vector.tensor_scalar_add(
                                out[:, j, c0:c0 + cw], ps[:, j, :],
                                ccol(bias_idx + j))
                return out

            def layernorm_z(zps_chunks, gcol, bcol, out_dt, out_tag,
                            store_cb=None, sub=None, grw=None):
                """LN from centered PSUM chunks: z -> z * rsqrt(E[z^2]+eps).
                zps_chunks: list of (c0, cw, psum tile [P,2,cw])."""
                out = xp.tile([P, 2, NC], out_dt, tag=out_tag, name=out_tag)
                zq = xp.tile([P, 2, NC], BF16, tag="zq", name="zq")
                for c0, cw, zps in zps_chunks:
                    nc.scalar.activation(
                        zq[:, :, c0:c0 + cw], zps,
                        mybir.ActivationFunctionType.Square)
                keepalive(3)
                ngr = NGR if grw is None else [
                    (g0, min(grw, NC - g0)) for g0 in range(0, NC, grw)]
                for g0, gw in ngr:
                    s2 = ps1.tile([1, gw], F32, tag="p1", name="s2")
                    for k in range(2):
                        nc.tensor.matmul(
                            s2[:, :], c256[:], zq[:, k, g0:g0 + gw],
                            start=(k == 0), stop=(k == 1),
                        )
                    keepalive(6)
                    lnv = rows.tile([1, gw], F32, tag="lnv")
                    nc.scalar.activation(
                        lnv, s2, mybir.ActivationFunctionType.Ln, bias=eps_row[:])
                    rstd = rows.tile([1, gw], BF16, tag="rstd")
                    nc.scalar.activation(
                        rstd, lnv, mybir.ActivationFunctionType.Exp, scale=-0.5)
                    rb = ps1.tile([P, gw], F32, tag="p1", name="rb")
                    nc.tensor.matmul(rb[:, :], ones_row[:], rstd[:],
                                     start=True, stop=True)
                    rb_sb = pp.tile([P, gw], BF16, tag="rb_sb", name="rb_sb")
                    nc.vector.tensor_copy(rb_sb, rb)
                    keepalive(4)
                    for c0, cw, zps in zps_chunks:
                        lo, hi = max(c0, g0), min(c0 + cw, g0 + gw)
                        if lo >= hi:
                            continue
                        step = sub or (hi - lo)
                        for p0 in range(lo, hi, step):
                            pw = min(step, hi - p0)
                            rba = repN(rb_sb[:, p0 - g0:p0 - g0 + pw], 2)
                            zc = zps[:, :, p0 - c0:p0 - c0 + pw]
                            dst = out[:, :, p0:p0 + pw]
                            if trivial_ln:
                                nc.vector.tensor_mul(dst, zc, rba)
                            else:
                                t2 = pp.tile([P, 2, pw], F32, tag="lnt",
                                             name="lnt")
                                nc.vector.tensor_mul(t2, zc, rba)
                                for j in range(2):
                                    nc.vector.tensor_scalar(
                                        out[:, j, p0:p0 + pw], t2[:, j, :],
                                        ccol(gcol + j), ccol(bcol + j),
                                        op0=mybir.AluOpType.mult,
                                        op1=mybir.AluOpType.add,
                                    )
                            if store_cb is not None:
                                store_cb(out, p0, pw)
                return out

            # ---------------------------------------------------------------
            # block 0 pre-layer: h0 = LN(tanh(x0 @ lin_W + lin_b))
            # ---------------------------------------------------------------
            t0 = xp.tile([P, 2, NC], BF16, tag="t0", name="t0")
            for c0, cw in NCH:
                ps = ps1.tile([P, 2, cw], F32, tag="p1", name="lin")
                for j in range(2):
                    for k in range(2):
                        nc.tensor.matmul(
                            ps[:, j, :], wcol(LIN_BASE, k, j),
                            x0[:, k, c0:c0 + cw],
                            start=(k == 0), stop=(k == 1),
                        )
                if trivial_b:
                    nc.scalar.activation(
                        t0[:, :, c0:c0 + cw], ps,
                        mybir.ActivationFunctionType.Tanh)
                else:
                    for j in range(2):
                        nc.scalar.activation(
                            t0[:, j, c0:c0 + cw], ps[:, j, :],
                            mybir.ActivationFunctionType.Tanh,
                            bias=ccol(c_lin("b", 0) + j))

            # mean row of t0 (tanh output is not centered)
            mean_row = rows.tile([1, NC], BF16, tag="meanr")
            for g0, gw in NGR:
                mps = ps1.tile([1, gw], F32, tag="p1", name="mean")
                for k in range(2):
                    nc.tensor.matmul(mps[:, :], c256[:], t0[:, k, g0:g0 + gw],
                                     start=(k == 0), stop=(k == 1))
                nc.vector.tensor_copy(mean_row[:, g0:g0 + gw], mps)
            # z = t0 - mean (identity matmul + rank-1 subtract, into PSUM)
            z0_chunks = []
            for c0, cw in NCH:
                zps = ps1.tile([P, 2, cw], F32, tag="p1", name="z0")
                for j in range(2):
                    nc.tensor.matmul(zps[:, j, :], id128(),
                                     t0[:, j, c0:c0 + cw],
                                     start=True, stop=False)
                    nc.tensor.matmul(zps[:, j, :], neg_row[:],
                                     mean_row[:, c0:c0 + cw],
                                     start=False, stop=True)
                z0_chunks.append((c0, cw, zps))
            h0 = layernorm_z(z0_chunks, c_lin("g", 0), c_lin("beta", 0),
                             BF16, "h0")

            # ---------------------------------------------------------------
            # attention blocks
            # ---------------------------------------------------------------
            def attention_block(i, hp, out_dt, out_tag,
                                store_cb=None, sub=None, grw=None):
                # K projection -> per-head masked SBUF tiles: head h keeps its
                # 32 partition rows, the rest are zeroed, so the score matmul
                # contracts over the full 128 partitions at base 0 (cross-head
                # terms multiply by zero).  Kills the SBUF->SBUF head shifts.
                kM = [xp.tile([P, NC], BF16, tag=f"kM{h}", name=f"kM{i}_{h}")
                      for h in range(H)]
                for c0, cw in NCH:
                    kps = ps1.tile([P, 2, cw], F32, tag="p1", name="kps")
                    for j in range(2):
                        for k in range(2):
                            nc.tensor.matmul(
                                kps[:, j, :],
                                wcol(w_base(i, "k"), k, j),
                                hp[:, k, c0:c0 + cw],
                                start=(k == 0), stop=(k == 1),
                            )
                    for h in range(H):
                        b, hh = divmod(h, 4)
                        dst = kM[h][:, c0:c0 + cw]
                        if not trivial_b:
                            nc.vector.tensor_scalar(
                                dst, kps[:, b, :],
                                ccol(c_blk(i, "k", 0) + b),
                                ccol(C_HMASK + hh),
                                op0=mybir.AluOpType.add,
                                op1=mybir.AluOpType.mult)
                        elif hh % 2 == 0:
                            nc.vector.tensor_scalar_mul(
                                dst, kps[:, b, :], ccol(C_HMASK + hh))
                        else:
                            nc.scalar.activation(
                                dst, kps[:, b, :],
                                mybir.ActivationFunctionType.Copy,
                                scale=ccol(C_HMASK + hh))

                qT = proj_chunks(hp, w_base(i, "q"), c_blk(i, "q", 0), f"qT{i}")

                def q_ap(b, hh, c0, cw):
                    return qT[:, b, c0:c0 + cw]

                def k_ap(b, hh, c0, cw):
                    return kM[4 * b + hh][:, c0:c0 + cw]

                def score_tp(hh):
                    return None
                # v in node layout: v[node, d] = hT_chunk.T @ Wv_chunk
                v_sb = xp.tile([P, T, 256], BF16, tag="v_all", name=f"v{i}")
                for t2i in range(0, T, 2):
                    npair = min(2, T - t2i)
                    vp = ps1.tile([P, npair, 256], F32, tag="p1", name="vp")
                    for tt in range(npair):
                        t = t2i + tt
                        for k in range(2):
                            nc.tensor.matmul(
                                vp[:, tt, :],
                                hp[:, k, t * P:(t + 1) * P],
                                wcol(w_base(i, "v"), k, 0, width=256),
                                start=(k == 0), stop=(k == 1),
                            )
                    nc.vector.tensor_copy(v_sb[:, t2i:t2i + npair, :], vp)

                o_sb = xp.tile([P, 2, NC], BF16, tag="o_sb", name=f"o{i}")

                # per-q-tile attention, software-pipelined: emit scores(qt)
                # ahead of pv(qt-1) so the PE always has ready work while
                # ACT runs the exp of the previous tile.
                state = {}

                def emit_scores(qi):
                    s, t_off, ts_s = qtiles[qi]
                    qc = qi * P
                    pts = []
                    for kt in range(ts_s):
                        ktg = t_off + kt
                        stp = ps2.tile([P, 2, 4, P], F32, tag="stp",
                                       name="stp")
                        for b in range(2):
                            for hh in range(4):
                                nc.tensor.matmul(
                                    stp[:, b, hh, :],
                                    k_ap(b, hh, ktg * P, P),
                                    q_ap(b, hh, qc, P),
                                    start=True, stop=True,
                                    tile_position=score_tp(hh),
                                )
                        p_t = pp.tile([P, 2, 4, P], BF16, tag="pT", name="pT")
                        nc.scalar.activation(
                            p_t, stp, mybir.ActivationFunctionType.Exp,
                            scale=SCALE, bias=ccol(C_FIXED + ktg))
                        pts.append(p_t)
                    state[qi] = pts

                def emit_pv(qi):
                    s, t_off, ts_s = qtiles[qi]
                    qc = qi * P
                    pts = state.pop(qi)
                    oT = ps1.tile([P, 2, P], F32, tag="p1", name="oT")
                    dnp = ps1.tile([P, 256], F32, tag="p1", name="dn")
                    for hh in range(4):
                        for kt in range(ts_s):
                            nc.tensor.matmul(
                                dnp[32 * hh:32 * hh + 32, :],
                                ones32[:],
                                pts[kt][:, :, hh, :],
                                start=(kt == 0), stop=(kt == ts_s - 1),
                                tile_position=(0, 32 * hh),
                            )
                    for b in range(2):
                        for hh in range(4):
                            for kt in range(ts_s):
                                nc.tensor.matmul(
                                    oT[32 * hh:32 * hh + 32, b, :],
                                    v_sb[:, t_off + kt,
                                         (4 * b + hh) * 32:(4 * b + hh) * 32 + 32],
                                    pts[kt][:, b, hh, :],
                                    start=(kt == 0), stop=(kt == ts_s - 1),
                                    tile_position=(0, 32 * hh),
                                )
                    r = pp.tile([P, 256], F32, tag="r", name="r")
                    nc.vector.reciprocal_approx_fast(out=r, in_=dnp)
                    nc.vector.tensor_mul(
                        o_sb[:, :, qc:qc + P], oT,
                        bass.AP(tensor=r.tensor, offset=r.offset,
                                ap=[list(r.ap[0]), [P, 2], [1, P]]))

                keepalive(3)
                emit_scores(0)
                for qi in range(1, n_qt):
                    emit_scores(qi)
                    emit_pv(qi - 1)
                keepalive(3)
                emit_pv(n_qt - 1)

                # output projection (centered WoC) + residual via identity MM
                z_chunks = []
                for c0, cw in NCH:
                    zps = ps1.tile([P, 2, cw], F32, tag="p1", name="zo")
                    for j in range(2):
                        nc.tensor.matmul(
                            zps[:, j, :], id128(), hp[:, j, c0:c0 + cw],
                            start=True, stop=False)
                        for k in range(2):
                            nc.tensor.matmul(
                                zps[:, j, :],
                                wcol(w_base(i, "o"), k, j),
                                o_sb[:, k, c0:c0 + cw],
                                start=False, stop=(k == 1 and trivial_b),
                            )
                        if not trivial_b:
                            nc.tensor.matmul(
                                zps[:, j, :],
                                bo_rows[:, (i * 2 + j) * 128:
                                        (i * 2 + j) * 128 + 128],
                                ones_wide(cw),
                                start=False, stop=True)
                    z_chunks.append((c0, cw, zps))
                return layernorm_z(z_chunks, c_blk(i, "g", 0),
                                   c_blk(i, "beta", 0), out_dt, out_tag,
                                   store_cb=store_cb, sub=sub, grw=grw)

            x1 = attention_block(0, h0, BF16, "x1")

            store_engs = [nc.sync, nc.scalar]
            store_ct = [0]

            def store_out(out, p0, pw):
                store_engs[store_ct[0] % 2].dma_start(
                    outT[:, :, p0:p0 + pw], out[:, :, p0:p0 + pw])
                store_ct[0] += 1

            attention_block(1, x1, F32, "x2", store_cb=store_out, sub=P,
                            grw=256)

    nc.finalize()
    return nc


# ---------------------------------------------------------------------------
# host side
# ---------------------------------------------------------------------------
_prog_cache = {}
_last_results = None


def _get_program(slot_ts, trivial_ln, trivial_b):
    key = (tuple(slot_ts), trivial_ln, trivial_b, DT_ACT_NAME,
           USE_WARMUP, USE_GPSIMD_SQ, SCORES_MODE)
    if key not in _prog_cache:
        _prog_cache[key] = build_program(tuple(slot_ts), trivial_ln, trivial_b)
    return _prog_cache[key]


def _segments(ordering):
    """contiguous runs of equal values in sorted ordering -> (start, len)."""
    n = ordering.shape[0]
    change = np.nonzero(np.diff(ordering))[0] + 1
    starts = np.concatenate([[0], change])
    lens = np.diff(np.concatenate([starts, [n]]))
    return list(zip(starts.tolist(), lens.tolist()))


def kernel(
    feat, ordering, lin_W, lin_b, lin_g, lin_beta,
    Wq, Wk, Wv, bq, bk, bv, Wo, bo, ln_g, ln_b,
):
    feat = np.asarray(feat, np.float32)
    ordering = np.asarray(ordering)
    N = feat.shape[0]
    np_act = _np_act()

    perm = np.argsort(ordering, kind="stable")
    segs = _segments(np.asarray(ordering)[perm])

    # deal segments (sorted by length desc) snake-wise to cores
    order = sorted(range(len(segs)), key=lambda i: -segs[i][1])
    core_slots = [[] for _ in range(N_CORES)]
    for r, si in enumerate(order):
        c = r % (2 * N_CORES)
        c = c if c < N_CORES else 2 * N_CORES - 1 - c
        core_slots[c].append(si)
    S = max(len(cs) for cs in core_slots)
    slot_ts = []
    for k in range(S):
        mx = 1
        for c in range(N_CORES):
            if k < len(core_slots[c]):
                mx = max(mx, (segs[core_slots[c][k]][1] + P - 1) // P)
        slot_ts.append(mx)
    T = sum(slot_ts)
    NC = T * P

    trivial_ln = bool(
        np.all(np.asarray(ln_g) == 1) and np.all(np.asarray(ln_b) == 0)
        and np.all(np.asarray(lin_g) == 1) and np.all(np.asarray(lin_beta) == 0)
    )
    # softmax rows sum to 1, so the value bias folds into the output bias:
    # bo_eff = bo + bv @ Wo.  The output projection is column-centered
    # (WoC = Wo - rowmean), which drops any per-node constant; center bo the
    # same way (constant offsets vanish inside the post-LN anyway).
    Wo = np.asarray(Wo, np.float32)
    bo_eff = np.asarray(bo, np.float32) + np.einsum(
        "id,idj->ij", np.asarray(bv, np.float32), Wo)
    WoC = Wo - Wo.mean(axis=2, keepdims=True)
    bo_c = bo_eff - bo_eff.mean(axis=1, keepdims=True)

    trivial_b = bool(
        np.all(np.asarray(lin_b) == 0) and np.all(np.asarray(bq) == 0)
        and np.all(np.asarray(bk) == 0) and np.all(np.abs(bo_c) < 1e-12)
    )
    nc = _get_program(slot_ts, trivial_ln, trivial_b)

    # ---- pack weights ----
    wallp = np.zeros((P, N_WCOLS), np.float32)

    def put_w(base, W):
        W = np.asarray(W, np.float32)
        for k in range(2):
            for j in range(2):
                c0 = base + k * 256 + j * 128
                wallp[:, c0:c0 + 128] = W[k * 128:(k + 1) * 128,
                                          j * 128:(j + 1) * 128]

    put_w(LIN_BASE, lin_W)
    wallp[:, ID_BASE:ID_BASE + 128] = np.eye(P, dtype=np.float32)
    for i in range(2):
        put_w(w_base(i, "q"), np.asarray(Wq)[i])
        put_w(w_base(i, "k"), np.asarray(Wk)[i])
        put_w(w_base(i, "v"), np.asarray(Wv)[i])
        put_w(w_base(i, "o"), WoC[i])
        wallp[0, BO_BASE + (i * 2) * 128:BO_BASE + (i * 2) * 128 + 256] = bo_c[i]
    wallp = wallp.astype(np_act)

    consp = np.zeros((P, C_FIXED + T), np.float32)

    def put_c(idx, vec):
        vec = np.asarray(vec, np.float32)
        consp[:, idx] = vec[:128]
        consp[:, idx + 1] = vec[128:]

    pidx = np.arange(P) // 32
    for hh in range(4):
        consp[:, C_HMASK + hh] = (pidx == hh).astype(np.float32)
    put_c(c_lin("b", 0), lin_b)
    put_c(c_lin("g", 0), lin_g)
    put_c(c_lin("beta", 0), lin_beta)
    for i in range(2):
        put_c(c_blk(i, "q", 0), np.asarray(bq)[i])
        put_c(c_blk(i, "k", 0), np.asarray(bk)[i])
        put_c(c_blk(i, "g", 0), np.asarray(ln_g)[i])
        put_c(c_blk(i, "beta", 0), np.asarray(ln_b)[i])

    # ---- per-core data ----
    feat_sorted = feat[perm]
    in_maps = []
    core_meta = []
    for c in range(N_CORES):
        fT = np.zeros((NC, 256), np.float32)
        mb = np.full((NC,), NEG, np.float32)
        meta = []
        off = 0
        for k in range(S):
            if k < len(core_slots[c]):
                st, ln = segs[core_slots[c][k]]
                fT[off:off + ln] = feat_sorted[st:st + ln]
                mb[off:off + ln] = 0.0
                meta.append((st, ln, off))
            off += slot_ts[k] * P
        cons_c = consp.copy()
        cons_c[:, C_FIXED:C_FIXED + T] = mb.reshape(T, P).T
        featT_c = np.ascontiguousarray(
            fT.T.reshape(2, P, NC).transpose(1, 0, 2)
        ).astype(np_act)
        in_maps.append({"featT": featT_c, "wall": wallp, "cons": cons_c})
        core_meta.append(meta)

    res = run_bass_kernel_spmd(nc, in_maps, list(range(N_CORES)))
    global _last_results
    _last_results = res

    out = np.empty((N, 256), np.float32)
    for c in range(N_CORES):
        oT = np.asarray(res.results[c]["outT"], np.float32)  # [128, 2, NC]
        o_nodes = oT.transpose(1, 0, 2).reshape(256, NC).T   # [NC, 256]
        for st, ln, off in core_meta[c]:
            out[perm[st:st + ln]] = o_nodes[off:off + ln]
    return out

